# revision 8
# baseline (speedup 1.0000x reference)
"""Trainium2 Bass kernel for nn_EquivariantDecoder.

Data-parallel over 8 NeuronCores (batch sharded, 2048 rows/core).

Host side packs v_raw into a feature-major bf16 layout ordered exactly
as the device consumes it, so the device does zero transposes and zero
casting DMAs: per b-tile three HWDGE loads (first one only 128KB so the
PE starts almost immediately), then the four e3linear layers run as
weight-stationary bf16 matmuls whose moving operand is the batch
dimension. Per-(l,m) blocks are packed two-deep in the contraction dim
(l3|l4, l5|l6, l1|l2) so most matmuls use the full 128 partitions. All
weights arrive in one DMA as a single [128, ~3.2K] blob. Gates run on
ACT (sigmoid/silu), gating multiplies on DVE with dense 2D APs (the
broadcast 3D form runs ~1.6x slower). The final layer accumulates all
49 outputs into one PSUM bank; results are stored feature-major
[49, BC] and transposed back on the host.
"""

import numpy as np
import ml_dtypes
from contextlib import ExitStack

import concourse.bass as bass
import concourse.mybir as mybir
import concourse.tile as tile
from concourse import bass_utils

BF16 = mybir.dt.bfloat16
FP32 = mybir.dt.float32
BF = ml_dtypes.bfloat16

# ---------------- problem constants (hardcoded) ----------------
B_FULL = 16384
NCORES = 8
BC = B_FULL // NCORES          # 2048 rows per core
BT = 512                       # b-tile
NT = BC // BT

IN_IRREPS = [(256, 0), (128, 1), (128, 2), (64, 3), (64, 4), (64, 5), (64, 6)]
HID_IRREPS = [(64, 0), (64, 1), (64, 2), (32, 3), (32, 4), (32, 5), (32, 6)]
N_SCALARS = 64
N_GATES = 256
D_IN = 3840
D_OUT = 49

IN_OFF = {}
_o = 0
for _mul, _l in IN_IRREPS:
    IN_OFF[_l] = _o
    _o += _mul * (2 * _l + 1)

OUT_OFF = {l: l * l for l in range(7)}

# gate channel permutation: [g_l2|g_l1 | g_l6|g_l5|g_l4|g_l3]
GPERM = ([64 + i for i in range(64)] + [i for i in range(64)] +
         [224 + i for i in range(32)] + [192 + i for i in range(32)] +
         [160 + i for i in range(32)] + [128 + i for i in range(32)])

# input group indices (device consumption order)
G_L0A, G_L0B = 0, 1
G_L1 = lambda m: 2 + m                 # m 0..2
G_L2 = lambda m: 5 + m                 # m 0..4
G_56 = lambda m: (10 + 2 * m) if m < 7 else {7: 24, 8: 25, 9: 27, 10: 28}[m]
G_34 = lambda m: 11 + 2 * m            # m 0..6
G_4H = 26                              # [l4 m7 | l4 m8]
G_6H = 29                              # [l6 m11 | l6 m12]

_BUILD = {}


def _build_P():
    """Feature permutation: 30 partition-blocks of 128 in device order."""
    blocks = {}
    blocks[G_L0A] = list(range(0, 128))
    blocks[G_L0B] = list(range(128, 256))
    for m in range(3):
        blocks[G_L1(m)] = [IN_OFF[1] + i * 3 + m for i in range(128)]
    for m in range(5):
        blocks[G_L2(m)] = [IN_OFF[2] + i * 5 + m for i in range(128)]
    for m in range(11):
        blocks[G_56(m)] = ([IN_OFF[5] + i * 11 + m for i in range(64)] +
                           [IN_OFF[6] + i * 13 + m for i in range(64)])
    for m in range(7):
        blocks[G_34(m)] = ([IN_OFF[3] + i * 7 + m for i in range(64)] +
                           [IN_OFF[4] + i * 9 + m for i in range(64)])
    blocks[G_4H] = ([IN_OFF[4] + i * 9 + 7 for i in range(64)] +
                    [IN_OFF[4] + i * 9 + 8 for i in range(64)])
    blocks[G_6H] = ([IN_OFF[6] + i * 13 + 11 for i in range(64)] +
                    [IN_OFF[6] + i * 13 + 12 for i in range(64)])
    P = []
    for g in range(30):
        P += blocks[g]
    assert len(P) == 3840 and len(set(P)) == 3840
    return np.array(P, np.int64)


P_FEAT = _build_P()


def _split_blocks(wflat, in_irr, out_irr):
    mul_in = {l: m for m, l in in_irr}
    blocks = []
    off = 0
    for mo, l in out_irr:
        mi = mul_in[l]
        w = wflat[off:off + mi * mo].reshape(mi, mo) / np.sqrt(mi)
        off += mi * mo
        blocks.append((l, w))
    assert off == wflat.size
    return blocks


def _pack_weights(w1, w2, w3, w4):
    out = {}
    pre = [(N_SCALARS, 0), (N_GATES, 0)] + [(m, l) for m, l in HID_IRREPS if l > 0]

    b1 = _split_blocks(w1, IN_IRREPS, pre)
    ws, wg = b1[0][1], b1[1][1]
    W10 = np.concatenate([ws, wg[:, GPERM]], axis=1)           # [256, 320]
    out["W1_0a"], out["W1_0b"] = W10[:128], W10[128:]
    wl = {l: w for l, w in b1[2:]}
    out["W1_l1"] = wl[1]                                       # [128, 64]
    out["W1_l2"] = wl[2]                                       # [128, 64]
    W134 = np.zeros((128, 64), np.float32)
    W134[0:64, 32:64] = wl[3]      # l3 -> psum 96:128 (out base 64)
    W134[64:128, 0:32] = wl[4]     # l4 -> psum 64:96
    out["W1_34"] = W134
    # duplicated across both partition halves: matmul requires stationary
    # and moving operands to start at the same partition index
    out["W1_l4"] = np.concatenate([wl[4], wl[4]], axis=0)      # [128, 32]
    W156 = np.zeros((128, 64), np.float32)
    W156[0:64, 32:64] = wl[5]      # l5 -> psum 32:64
    W156[64:128, 0:32] = wl[6]     # l6 -> psum 0:32
    out["W1_56"] = W156
    out["W1_l6"] = np.concatenate([wl[6], wl[6]], axis=0)      # [128, 32]

    for name, wflat in (("W2", w2), ("W3", w3)):
        b = _split_blocks(wflat, HID_IRREPS, pre)
        ws, wg = b[0][1], b[1][1]
        out[name + "_0"] = np.concatenate([ws, wg[:, GPERM]], axis=1)  # [64,320]
        wl = {l: w for l, w in b[2:]}
        W12 = np.zeros((128, 128), np.float32)
        W12[0:64, 0:64] = wl[2]
        W12[64:128, 64:128] = wl[1]
        out[name + "_12"] = W12
        out[name + "_l2"] = wl[2]                              # [64, 64]
        WB4 = np.zeros((128, 128), np.float32)
        for j, l in enumerate((6, 5, 4, 3)):
            WB4[32 * j:32 * (j + 1), 32 * j:32 * (j + 1)] = wl[l]
        out[name + "_B4"] = WB4

    b4 = _split_blocks(w4, HID_IRREPS, [(1, l) for l in range(7)])
    w4l = {l: w[:, 0] for l, w in b4}
    W4B = np.zeros((128, 13 * D_OUT), np.float32)
    for l in (3, 4, 5, 6):
        pd = 32 * (6 - l)
        for m in range(2 * l + 1):
            W4B[pd:pd + 32, m * D_OUT + OUT_OFF[l] + m] = w4l[l][:, None][:, 0]
    out["W4_B"] = W4B
    W4A = np.zeros((128, 5 * D_OUT), np.float32)
    for m in range(5):
        W4A[0:64, m * D_OUT + OUT_OFF[2] + m] = w4l[2]
    for m in range(3):
        W4A[64:128, m * D_OUT + OUT_OFF[1] + m] = w4l[1]
    out["W4_A"] = W4A
    W40 = np.zeros((64, D_OUT), np.float32)
    W40[:, 0] = w4l[0]
    out["W4_0"] = W40

    # ---- assemble single blob [128, WCOLS] ----
    cols = sum(a.shape[1] for a in out.values())
    blob = np.zeros((128, cols), np.float32)
    off = 0
    offs = {}
    for name in _WNAMES:
        a = out[name]
        p, n = a.shape
        blob[0:p, off:off + n] = a
        offs[name] = (p, off, n)
        off += n
    return blob.astype(BF), offs


_WNAMES = ["W1_0a", "W1_0b", "W1_l1", "W1_l2", "W1_34", "W1_l4", "W1_56",
           "W1_l6", "W2_0", "W2_12", "W2_l2", "W2_B4",
           "W3_0", "W3_12", "W3_l2", "W3_B4", "W4_B", "W4_A", "W4_0"]
_WSHAPES = {
    "W1_0a": (128, 320), "W1_0b": (128, 320), "W1_l1": (128, 64),
    "W1_l2": (128, 64), "W1_34": (128, 64), "W1_l4": (128, 32),
    "W1_56": (128, 64), "W1_l6": (128, 32),
    "W2_0": (64, 320), "W2_12": (128, 128), "W2_l2": (64, 64),
    "W2_B4": (128, 128),
    "W3_0": (64, 320), "W3_12": (128, 128), "W3_l2": (64, 64),
    "W3_B4": (128, 128),
    "W4_B": (128, 13 * D_OUT), "W4_A": (128, 5 * D_OUT), "W4_0": (64, D_OUT),
}
_WCOLS = sum(n for _, n in _WSHAPES.values())


def _split_excess_waits(nc, max_waits=1):
    """This walrus build accepts only one sem-wait per instruction on
    some ops; hoist excess waits onto same-engine NoOps inserted before."""
    for f in nc.m.functions:
        for bb in f.blocks:
            newlist = []
            changed = False
            for ins in bb.instructions:
                si = ins.sync_info
                waits = list(si.on_wait) if (si and si.on_wait) else []
                if len(waits) > max_waits:
                    extras, keep = waits[:-max_waits], waits[-max_waits:]
                    for k in range(0, len(extras), max_waits):
                        nop = mybir.InstNoOp(
                            name=f"{ins.name}_waitnop{k}", ins=[], outs=[],
                            engine=ins.engine)
                        nop.sync_info = mybir.SyncInfo(
                            on_wait=extras[k:k + max_waits], on_update=[])
                        nc.register_instruction(nop)
                        newlist.append(nop)
                    ins.sync_info = mybir.SyncInfo(
                        on_wait=keep,
                        on_update=list(si.on_update) if si.on_update else [])
                    changed = True
                newlist.append(ins)
            if changed:
                bb.instructions[:] = newlist
    return nc


def _build_program():
    nc = bass.Bass("TRN2", target_bir_lowering=False, debug=False)

    vt = nc.dram_tensor("vt", [NT, 128, 30, BT], BF16, kind="ExternalInput").ap()
    wb = nc.dram_tensor("wb", [128, _WCOLS], BF16, kind="ExternalInput").ap()
    out49 = nc.dram_tensor("out49", [D_OUT, BC], FP32, kind="ExternalOutput").ap()

    with tile.TileContext(nc) as tc:
        with ExitStack() as ctx:
            _emit(ctx, tc, nc, vt, wb, out49)

    _split_excess_waits(nc)
    return nc


def _emit(ctx, tc, nc, vt, wb, out49):
    mm = nc.tensor.matmul
    Sig = mybir.ActivationFunctionType.Sigmoid
    Silu = mybir.ActivationFunctionType.Silu

    wpool = ctx.enter_context(tc.tile_pool(name="weights", bufs=1))
    vpool = ctx.enter_context(tc.tile_pool(name="vtiles", bufs=2))
    hpool = ctx.enter_context(tc.tile_pool(name="htiles", bufs=2))
    gpool = ctx.enter_context(tc.tile_pool(name="gates", bufs=2))
    opool = ctx.enter_context(tc.tile_pool(name="outs", bufs=2))
    zpool = ctx.enter_context(tc.tile_pool(name="zb", bufs=2, space="PSUM"))
    z4pool = ctx.enter_context(tc.tile_pool(name="z4", bufs=2, space="PSUM"))

    wtile = wpool.tile([128, _WCOLS], BF16, tag="wb")
    nc.sync.dma_start(out=wtile, in_=wb)
    W = {}
    off = 0
    for name in _WNAMES:
        p, n = _WSHAPES[name]
        v = wtile[0:p, off:off + n]
        if name in ("W4_B", "W4_A"):
            v = v.rearrange("p (m n) -> p m n", n=D_OUT)
        W[name] = v
        off += n

    # valid-partition prefix of the B z-tile per m (layout l6|l5|l4|l3)
    def bphi(m):
        return 128 if m < 7 else (96 if m < 9 else (64 if m < 11 else 32))

    for t in range(NT):
        vtile = vpool.tile([128, 30, BT], BF16, tag="vt")
        # split load in consumption order so the PE starts early
        nc.sync.dma_start(out=vtile[:, 0:2, :], in_=vt[t, :, 0:2, :])
        nc.sync.dma_start(out=vtile[:, 2:10, :], in_=vt[t, :, 2:10, :])
        nc.sync.dma_start(out=vtile[:, 10:30, :], in_=vt[t, :, 10:30, :])

        x = lambda g: vtile[:, g, :]
        xh = lambda g, h: vtile[64 * h:64 * (h + 1), g, :]

        def gates_from_z0(z0):
            h0 = hpool.tile([64, BT], BF16, tag="h0")
            gA = gpool.tile([128, BT], BF16, tag="gA")
            gB = gpool.tile([128, BT], BF16, tag="gB")
            nc.scalar.activation(h0, z0[0:64, 0, :], Silu)
            nc.scalar.activation(gA, z0[:, 1, :], Sig)
            nc.scalar.activation(gB, z0[:, 2, :], Sig)
            return h0, gA, gB

        def gate_A(zA1, zA2, gA):
            hA = hpool.tile([128, 5, BT], BF16, tag="hA")
            for s in range(3):
                nc.vector.tensor_mul(hA[:, s, :], zA1[:, s, :], gA)
            for s in range(2):
                nc.vector.tensor_mul(hA[0:64, 3 + s, :], zA2[0:64, s, :],
                                     gA[0:64, :])
            return hA

        def gate_B(hB, zB, mlo, nm, gB, phis):
            for j in range(nm):
                phi = phis[j]
                nc.vector.tensor_mul(hB[0:phi, mlo + j, :], zB[0:phi, j, :],
                                     gB[0:phi, :])

        # ---------------- layer 1 ----------------
        z0 = zpool.tile([128, 3, BT], FP32, tag="zb")
        for s, (wk, g) in enumerate(((W["W1_0a"], G_L0A), (W["W1_0b"], G_L0B))):
            st, sp = (s == 0), (s == 1)
            mm(z0[0:64, 0, :], wk[:, 0:64], x(g), start=st, stop=sp)
            mm(z0[:, 1, :], wk[:, 64:192], x(g), start=st, stop=sp)
            mm(z0[:, 2, :], wk[:, 192:320], x(g), start=st, stop=sp)
        h0, gA, gB = gates_from_z0(z0)

        zA1 = zpool.tile([128, 3, BT], FP32, tag="zb")
        for m in range(3):
            mm(zA1[0:64, m, :], W["W1_l2"], x(G_L2(m)), start=True, stop=True,
               tile_position=(0, 0))
            mm(zA1[64:128, m, :], W["W1_l1"], x(G_L1(m)), start=True, stop=True,
               tile_position=(0, 64))
        zA2 = zpool.tile([128, 3, BT], FP32, tag="zb")
        for j, m in enumerate((3, 4)):
            mm(zA2[0:64, j, :], W["W1_l2"], x(G_L2(m)), start=True, stop=True,
               tile_position=(0, 0))
        hA = gate_A(zA1, zA2, gA)

        hB = hpool.tile([128, 13, BT], BF16, tag="hB")
        for mlo in (0, 3, 6, 9, 12):
            nm = min(3, 13 - mlo)
            zB = zpool.tile([128, 3, BT], FP32, tag="zb")
            phis = []
            for j in range(nm):
                m = mlo + j
                if m < 7:
                    mm(zB[0:64, j, :], W["W1_56"], x(G_56(m)), start=True,
                       stop=True, tile_position=(0, 0))
                    mm(zB[64:128, j, :], W["W1_34"], x(G_34(m)), start=True,
                       stop=True, tile_position=(0, 64))
                elif m < 9:
                    h = m - 7
                    mm(zB[0:64, j, :], W["W1_56"], x(G_56(m)), start=True,
                       stop=True, tile_position=(0, 0))
                    mm(zB[64:96, j, :], W["W1_l4"][64 * h:64 * (h + 1), :],
                       xh(G_4H, h), start=True, stop=True,
                       tile_position=(64 * h, 64))
                elif m < 11:
                    mm(zB[0:64, j, :], W["W1_56"], x(G_56(m)), start=True,
                       stop=True, tile_position=(0, 0))
                else:
                    h = m - 11
                    mm(zB[0:32, j, :], W["W1_l6"][64 * h:64 * (h + 1), :],
                       xh(G_6H, h), start=True, stop=True,
                       tile_position=(64 * h, 0))
                phis.append(bphi(m))
            gate_B(hB, zB, mlo, nm, gB, phis)

        # ---------------- layers 2, 3 ----------------
        for ln in ("W2", "W3"):
            w0, w12, wl2, wb4 = W[ln + "_0"], W[ln + "_12"], W[ln + "_l2"], W[ln + "_B4"]
            z0 = zpool.tile([128, 3, BT], FP32, tag="zb")
            mm(z0[0:64, 0, :], w0[:, 0:64], h0, start=True, stop=True)
            mm(z0[:, 1, :], w0[:, 64:192], h0, start=True, stop=True)
            mm(z0[:, 2, :], w0[:, 192:320], h0, start=True, stop=True)
            nh0, gA, gB = gates_from_z0(z0)

            zA1 = zpool.tile([128, 3, BT], FP32, tag="zb")
            for m in range(3):
                mm(zA1[:, m, :], w12, hA[:, m, :], start=True, stop=True,
                   tile_position=(0, 0))
            zA2 = zpool.tile([128, 3, BT], FP32, tag="zb")
            for j, m in enumerate((3, 4)):
                mm(zA2[0:64, j, :], wl2, hA[0:64, m, :], start=True, stop=True,
                   tile_position=(0, 0))
            nhA = gate_A(zA1, zA2, gA)

            nhB = hpool.tile([128, 13, BT], BF16, tag="hB")
            for mlo in (0, 3, 6, 9, 12):
                nm = min(3, 13 - mlo)
                zB = zpool.tile([128, 3, BT], FP32, tag="zb")
                phis = []
                for j in range(nm):
                    m = mlo + j
                    kp = bphi(m)
                    mm(zB[0:kp, j, :], wb4[0:kp, 0:kp], hB[0:kp, m, :],
                       start=True, stop=True, tile_position=(0, 0))
                    phis.append(kp)
                gate_B(nhB, zB, mlo, nm, gB, phis)

            h0, hA, hB = nh0, nhA, nhB

        # ---------------- layer 4 ----------------
        z4 = z4pool.tile([D_OUT, BT], FP32, tag="z4")
        mm(z4, W["W4_0"], h0, start=True, stop=False, tile_position=(0, 0))
        for m in range(5):
            kp = 128 if m < 3 else 64
            mm(z4, W["W4_A"][0:kp, m, :], hA[0:kp, m, :], start=False,
               stop=False, tile_position=(0, 0))
        for m in range(13):
            kp = bphi(m)
            mm(z4, W["W4_B"][0:kp, m, :], hB[0:kp, m, :], start=False,
               stop=(m == 12), tile_position=(0, 0))

        z4sb = opool.tile([D_OUT, BT], FP32, tag="z4sb")
        nc.scalar.copy(out=z4sb, in_=z4)
        nc.sync.dma_start(out=out49[:, t * BT:(t + 1) * BT], in_=z4sb)


def _get_nc():
    if "nc" not in _BUILD:
        _BUILD["nc"] = _build_program()
    return _BUILD["nc"]


def kernel(v_raw, w1, w2, w3, w4):
    nc = _get_nc()
    wblob, _offs = _pack_weights(np.asarray(w1), np.asarray(w2),
                                 np.asarray(w3), np.asarray(w4))
    v_raw = np.asarray(v_raw, dtype=np.float32)
    vP = v_raw[:, P_FEAT].astype(BF)                 # [B, 3840] feature-permuted
    in_maps = []
    for c in range(NCORES):
        sl = vP[c * BC:(c + 1) * BC]                 # [BC, 3840]
        vt = np.ascontiguousarray(
            sl.reshape(NT, BT, 30, 128).transpose(0, 3, 2, 1))
        in_maps.append({"vt": vt, "wb": wblob})
    res = bass_utils.run_bass_kernel_spmd(nc, in_maps, core_ids=list(range(NCORES)))
    outs = [res.results[c]["out49"] for c in range(NCORES)]   # [49, BC] each
    full = np.concatenate([o.T for o in outs], axis=0)        # [B, 49]
    return np.ascontiguousarray(full).reshape(B_FULL, D_OUT, 1).astype(np.float32)
